# revision 18
# baseline (speedup 1.0000x reference)
"""Causal self-attention (non-masked softmax path) for TRN2, 8 NeuronCores.

Sharding: 2-way data parallel over batch x 4-way tensor parallel over heads.
Core c handles batch b = c // 4, head group g = c % 4 (heads 4g..4g+3).
Each core computes its QKV projection slice, full attention for its 4 heads,
and the row-parallel c_proj partial; the host sums the 4 partials per batch
(the all-reduce of row-parallel tensor parallelism) and adds b_proj.

The schedule is built around the ScalarE exp roofline: softmax exp is
16.8M elements/core at 1 elem/cycle/lane @1.2GHz = ~142us, which exceeds
the PE matmul issue floor (~130us). So ACT runs *only* exp, back-to-back,
and every other engine's work is interleaved around it:

 - QKV projection and c_proj matmuls are emitted as "filler" units spread
   between attention blocks so the PE stays busy without ever making ACT
   wait (no serial projection phase up front, no c_proj phase at the end).
 - All PSUM->SBUF evacuation runs on DVE (plain tensor_copy); q/k biases
   are applied with K=1 rank-1 matmuls appended to the projection chains,
   v bias likewise (ones-row trick), so no ACT Identity copies remain.
 - PV is col-packed: 2 heads per PE pass (M=64 each at col groups 0/64),
   softmax denominators come from 4 col-packed M=1 ones-matmuls (one per
   32-wide col group) accumulated in a dedicated psum bank, landing at
   partitions {0,32,64,96} exactly where the reciprocal/broadcast wants
   them. Denominator matmuls are emitted 2 blocks late so the dg bank's
   reuse dependency (prev j's reciprocal) never stalls the PE FIFO.
 - 1/d is broadcast across the 64 head-dim partitions with a single
   selector matmul per head pair (sel2), then applied by DVE tensor_mul
   straight out of PSUM.

All matmuls take bf16 inputs (fp32 matmul on TRN2 runs 4x slower) and
accumulate in fp32 PSUM. c_proj partials are DMA'd out in bf16 (halves
output traffic; the host sums in fp32). Measured end-to-end error vs the
fp32 reference is ~7e-3 scale-relative.

PSUM budget (8 banks): S tiles 2x[128,1024] (4) | PV accum 2x[128,512]
(2, heads pair-packed in partitions) | denominators 1x[128,512] (1) |
streaming proj/c_proj/bc tile 1x[128,512] (1).
"""

import numpy as np

B, T, H, NH, HD = 2, 2048, 1024, 16, 64
P = 128
FG = 256          # features per head group (4 heads x 64)
NQ = 512          # Tq chunk (psum free dim)
NJ = T // NQ      # 4
NI = T // P       # 16 key chunks
KH = H // P       # 8 hidden chunks
NCORES = 8
W3 = 3 * FG       # 768, wqkv row width
DDELAY = 3        # blocks by which denominator matmuls trail their E tile

_CACHE = {}


def _build():
    import concourse.bacc as bacc
    import concourse.mybir as mybir
    import concourse.tile as tile

    fp32 = mybir.dt.float32
    bf16 = mybir.dt.bfloat16

    nc = bacc.Bacc("TRN2", debug=False)
    xT = nc.dram_tensor("xT", [H, T], bf16, kind="ExternalInput").ap()
    wqkv = nc.dram_tensor("wqkv", [H, W3], bf16, kind="ExternalInput").ap()
    bqk = nc.dram_tensor("bqk", [2 * FG], bf16, kind="ExternalInput").ap()
    bv = nc.dram_tensor("bv", [FG], bf16, kind="ExternalInput").ap()
    wp = nc.dram_tensor("wp", [FG, H], bf16, kind="ExternalInput").ap()
    out = nc.dram_tensor("out", [T, H], bf16, kind="ExternalOutput").ap()

    with tile.TileContext(nc) as tc:
        _emit(nc, tc, mybir, xT, wqkv, bqk, bv, wp, out)
    nc.compile()
    return nc


def _emit(nc, tc, mybir, xT, wqkv, bqk, bv, wp, out):
    from contextlib import ExitStack

    fp32 = mybir.dt.float32
    bf16 = mybir.dt.bfloat16
    Exp = mybir.ActivationFunctionType.Exp

    with ExitStack() as ctx:
        pool = lambda name, bufs=1, space="SBUF": ctx.enter_context(
            tc.tile_pool(name=name, bufs=bufs, space=space)
        )

        const = pool("const")
        # warm the exp table before any real dependency exists
        wt = const.tile([1, 16], bf16)
        nc.vector.memset(wt[:], 0.0)
        we = const.tile([1, 16], bf16)
        nc.scalar.activation(we[:], wt[:], Exp)

        ones = const.tile([1, P], bf16)       # K=1 lhsT for v-bias broadcast
        nc.vector.memset(ones[:], 1.0)
        onesq = const.tile([1, NQ], bf16)     # K=1 rhs for qk-bias broadcast
        nc.vector.memset(onesq[:], 1.0)
        ones_col = const.tile([P, 1], bf16)   # denominator lhsT (M=1)
        nc.vector.memset(ones_col[:], 1.0)
        # sel2[k, p*128 + m] = 1 iff k == 32*(2p + (m>=64)); sel2-slice.T @ rcp
        # replicates head (2p+hh)'s 1/d row across its 64 y^T partitions.
        sel2 = const.tile([P, 2 * P], bf16)
        nc.vector.memset(sel2[:], 0.0)
        for pp in range(2):
            for hh in range(2):
                r = 32 * (2 * pp + hh)
                nc.vector.memset(
                    sel2[r : r + 1, pp * P + hh * 64 : pp * P + (hh + 1) * 64], 1.0
                )
        bqk_sb = const.tile([1, 2 * FG], bf16)
        nc.sync.dma_start(bqk_sb[:], bqk.rearrange("(o f) -> o f", o=1))
        bv_sb = const.tile([1, FG], bf16)
        nc.sync.dma_start(bv_sb[:], bv.rearrange("(o f) -> o f", o=1))

        xt_sb = pool("xt").tile([P, KH * T], bf16)
        w_sb = pool("w").tile([P, KH * W3], bf16)
        wp_sb = pool("wp").tile([P, 2 * H], bf16)

        def dma_w(k, lo, hi):
            nc.sync.dma_start(
                w_sb[:, k * W3 + lo : k * W3 + hi],
                wqkv[k * P : (k + 1) * P, lo:hi],
            )

        def dma_x(k, jq):
            nc.sync.dma_start(
                xt_sb[:, k * T + jq * NQ : k * T + (jq + 1) * NQ],
                xT[k * P : (k + 1) * P, jq * NQ : (jq + 1) * NQ],
            )

        # prefetch in first-use order: the m=2 (k0) weight slice + xT block 0
        # gate the very first projection chain, then the other qk slices in
        # chain order, then v weights, then the remaining xT blocks.
        for k in range(KH):
            dma_w(k, 2 * P, 3 * P)
            dma_x(k, 0)
        for lo, hi in ((0, P), (3 * P, 4 * P), (P, 2 * P), (2 * FG, W3)):
            for k in range(KH):
                dma_w(k, lo, hi)
        for jq in range(1, NJ):
            for k in range(KH):
                dma_x(k, jq)
        for kk in range(2):
            nc.sync.dma_start(
                wp_sb[:, kk * H : (kk + 1) * H], wp[kk * P : (kk + 1) * P, :]
            )

        qk_sb = pool("qk").tile([P, 4 * T], bf16)   # feat chunks: q0 q1 k0 k1
        v_sb = pool("v").tile([P, NI * FG], bf16)   # [p, (t h c)] c=64, no pad
        v4 = v_sb.rearrange("p (t h c) -> p t h c", t=NI, h=4, c=64)
        y_sb = pool("y").tile([P, 2 * T], bf16)     # y^T, feat pair chunks x T

        # one psum pool, 8 banks exactly:
        #   tag s  = 2 x [128,1024] (4 banks)  S^T pair tiles -> exp
        #   tag pv = 2 x [128, 512] (2 banks)  PV accum, 2 heads col-packed
        #   tag dg = 1 x [128, 512] (1 bank)   denominators at rows {0,32,64,96}
        #   tag st = 1 x [128, 512] (1 bank)   streaming proj/c_proj/bc tile
        ps_pool = ctx.enter_context(tc.tile_pool(name="ps", bufs=2, space="PSUM"))
        epool = pool("e", bufs=8)  # E lifetime spans DDELAY+1 blocks x 2 tiles
        pvs_pool = pool("pvs", bufs=2)
        rcp_pool = pool("rcp", bufs=2)
        outp = pool("outp", bufs=3)

        # ---------- filler work units (run on PE/DVE between attention) ----
        # Units are split to ~1us of PE time each so a popped unit never
        # delays the next attention block's S matmuls by more than the
        # per-block ACT slack. Two-part units (a/b) hold their "st" psum
        # slot between parts; the flat per-j queues keep a/b adjacent so no
        # other "st" user can interleave.
        def qk_chain_a(m, jq):
            ps = ps_pool.tile([P, NQ], fp32, tag="st", bufs=1, name=f"qk{m}_{jq}")
            state[("qkps", m, jq)] = ps
            for k in range(KH // 2):
                nc.tensor.matmul(
                    ps[:],
                    w_sb[:, k * W3 + m * P : k * W3 + (m + 1) * P],
                    xt_sb[:, k * T + jq * NQ : k * T + (jq + 1) * NQ],
                    start=(k == 0),
                    stop=False,
                )

        def qk_chain_b(m, jq):
            ps = state.pop(("qkps", m, jq))
            for k in range(KH // 2, KH):
                nc.tensor.matmul(
                    ps[:],
                    w_sb[:, k * W3 + m * P : k * W3 + (m + 1) * P],
                    xt_sb[:, k * T + jq * NQ : k * T + (jq + 1) * NQ],
                    start=False,
                    stop=False,
                )
            nc.tensor.matmul(  # += b[m-chunk] outer ones (bias over queries)
                ps[:],
                bqk_sb[0:1, m * P : (m + 1) * P],
                onesq[0:1, :],
                start=False,
                stop=True,
            )
            nc.vector.tensor_copy(
                qk_sb[:, m * T + jq * NQ : m * T + (jq + 1) * NQ], ps[:]
            )

        def _v_half(ps, tt, half):
            t = 2 * tt + half
            for k in range(KH):
                nc.tensor.matmul(
                    ps[:, half * FG : (half + 1) * FG],
                    xt_sb[:, k * T + t * P : k * T + (t + 1) * P],
                    w_sb[:, k * W3 + 2 * FG : (k + 1) * W3],
                    start=(k == 0),
                    stop=False,
                )
            nc.tensor.matmul(  # += ones.T @ bv (bias broadcast over rows)
                ps[:, half * FG : (half + 1) * FG],
                ones[0:1, :],
                bv_sb[0:1, :],
                start=False,
                stop=True,
            )

        def v_chain_a(tt):
            ps = ps_pool.tile([P, NQ], fp32, tag="st", bufs=1, name=f"v{tt}")
            state[("vps", tt)] = ps
            _v_half(ps, tt, 0)

        def v_chain_b(tt):
            ps = state.pop(("vps", tt))
            _v_half(ps, tt, 1)
            nc.vector.tensor_copy(v_sb[:, 2 * tt * FG : (2 * tt + 2) * FG], ps[:])

        def cproj_chain(mq, n, tag="st"):
            ps = ps_pool.tile(
                [P, NQ], fp32, tag=tag, bufs=(2 if tag == "pv" else 1),
                name=f"c{mq}_{n}",
            )
            for kk in range(2):
                nc.tensor.matmul(
                    ps[:],
                    y_sb[:, kk * T + mq * P : kk * T + (mq + 1) * P],
                    wp_sb[:, kk * H + n * NQ : kk * H + (n + 1) * NQ],
                    start=(kk == 0),
                    stop=(kk == 1),
                )
            ot = outp.tile([P, NQ], bf16, tag="o")
            nc.vector.tensor_copy(ot[:], ps[:])
            nc.sync.dma_start(out[mq * P : (mq + 1) * P, n * NQ : (n + 1) * NQ], ot[:])

        # ---------- per-j normalize state ----------
        state = {}

        def bc_mul(j, p):
            # broadcast 1/d across 64 partitions per head (one selector MM),
            # then y^T = pv * bc on DVE straight out of psum.
            rcp_t = state[("rcp", j)]
            pvs = state[("pvs", j, p)]
            bc = ps_pool.tile([P, NQ], fp32, tag="st", bufs=1, name=f"bc{j}_{p}")
            nc.tensor.matmul(
                bc[:], sel2[:, p * P : (p + 1) * P], rcp_t[:], start=True, stop=True
            )
            nc.vector.tensor_mul(
                y_sb[:, p * T + j * NQ : p * T + (j + 1) * NQ], pvs[:], bc[:]
            )

        # ---------- attention block emission ----------
        def emit_block(j, i):
            # delayed denominator matmuls for block (j, i-DDELAY)
            di = i - DDELAY
            if di >= 0:
                if di == 0:
                    dg = ps_pool.tile([P, NQ], fp32, tag="dg", bufs=1, name=f"dg{j}")
                    state[("dg", j)] = dg
                    # junk rows stay finite (1.0) so reciprocal never sees 0;
                    # accumulated rows start from 0.0 so the h>0 chains (which
                    # may accumulate-onto rather than overwrite, depending on
                    # how start's has_written clear scopes) are exact.
                    nc.vector.memset(dg[:], 1.0)
                    for h in (1, 2, 3):
                        nc.vector.memset(dg[32 * h : 32 * h + 1, :], 0.0)
                _denoms(j, di)
            es = []
            for p in range(2):
                sp = ps_pool.tile([P, 2 * NQ], fp32, tag="s", name=f"s{j}_{i}_{p}")
                for hh in range(2):
                    bp = 64 * hh
                    nc.tensor.matmul(  # S^T chunk, K=64 row-packed x2
                        sp[:, hh * NQ : (hh + 1) * NQ],
                        qk_sb[
                            bp : bp + 64,
                            (2 + p) * T + i * P : (2 + p) * T + (i + 1) * P,
                        ],
                        qk_sb[bp : bp + 64, p * T + j * NQ : p * T + (j + 1) * NQ],
                        start=True,
                        stop=True,
                        tile_position=(bp, 0),
                    )
                e = epool.tile([P, 2 * NQ], bf16, tag="e")
                nc.scalar.activation(e[:], sp[:], Exp)
                es.append(e)
                state[("e", j, i, p)] = e
            for p in range(2):
                pv = state[("pv", j, p)]
                for hh in range(2):
                    # 2 heads col-packed per pass share one psum bank, so they
                    # form ONE accumulation group: start clears has_written
                    # bank-wide; later first-writes overwrite-where-unwritten.
                    nc.tensor.matmul(
                        pv[64 * hh : 64 * (hh + 1), :],
                        v4[:, i, 2 * p + hh, :],
                        es[p][:, hh * NQ : (hh + 1) * NQ],
                        start=(i == 0 and hh == 0),
                        stop=(i == NI - 1 and hh == 1),
                        tile_position=(0, 64 * hh),
                        skip_group_check=True,
                    )

        def _denoms(j, di):
            dg = state[("dg", j)]
            for h in range(4):
                p, hh = divmod(h, 2)
                e = state.pop(("e", j, di, p)) if hh == 1 else state[("e", j, di, p)]
                nc.tensor.matmul(  # d[32h, q] += sum_k E[k, q]; M=1 col-packed
                    dg[32 * h : 32 * h + 1, :],
                    ones_col[:, 0:1],
                    e[:, hh * NQ : (hh + 1) * NQ],
                    start=(di == 0 and h == 0),
                    stop=(di == NI - 1 and h == 3),
                    tile_position=(0, 32 * h),
                    skip_group_check=True,
                )

        def alloc_pv(j):
            for p in range(2):
                pv = ps_pool.tile([P, NQ], fp32, tag="pv", bufs=2, name=f"pv{j}_{p}")
                state[("pv", j, p)] = pv
                # partitions 64-127 are first written by a start=False matmul;
                # zero them so accumulate-onto-stale is exact either way.
                nc.vector.memset(pv[64:128, :], 0.0)

        def finish_j(j):
            # trailing denominator matmuls (no delay needed past block 15),
            # then stage PV psum to SBUF (frees pv banks) + reciprocal.
            for di in range(NI - DDELAY, NI):
                _denoms(j, di)
            for p in range(2):
                pvs = pvs_pool.tile([P, NQ], fp32, tag="pvs", name=f"pvs{j}_{p}")
                nc.vector.tensor_copy(pvs[:], state.pop(("pv", j, p))[:])
                state[("pvs", j, p)] = pvs
            # next j's pv memsets go on the DVE queue BEFORE the (slow)
            # reciprocal, else PV(j+1, i=0) stalls the PE FIFO ~4us and the
            # HAM re-throttles the clock at every j boundary.
            if j + 1 < NJ:
                alloc_pv(j + 1)
            rcp_t = rcp_pool.tile([P, NQ], bf16, tag="rcp")
            with nc.allow_low_precision(reason="softmax denom broadcast in bf16"):
                nc.vector.reciprocal(rcp_t[:], state.pop(("dg", j))[:])
            state[("rcp", j)] = rcp_t

        # ---------- filler schedule ----------
        # Per-j ordered unit queues + per-block pop counts. Order guarantees
        # dependencies (a before b, producers a couple of blocks before
        # consumers) and keeps "st"-slot users strictly sequential.
        def qk2(m, jq):
            return [lambda: qk_chain_a(m, jq), lambda: qk_chain_b(m, jq)]

        def vc2(tt):
            return [lambda: v_chain_a(tt), lambda: v_chain_b(tt)]

        queues = {
            0: (
                vc2(1) + qk2(2, 1) + qk2(3, 1) + vc2(2) + vc2(3)
                + qk2(2, 2) + qk2(3, 2) + vc2(4) + vc2(5)
                + qk2(2, 3) + qk2(3, 3) + vc2(6) + vc2(7)
                + qk2(0, 1) + qk2(1, 1)
            ),
        }
        pops = {0: [2] * 15 + [0]}
        for j in range(1, NJ):
            pj = j - 1
            q = [lambda pj=pj: bc_mul(pj, 0), lambda pj=pj: bc_mul(pj, 1)]
            for u in range(8):
                mq, n = divmod(u, 2)
                q.append(lambda pj=pj, mq=mq, n=n: cproj_chain(4 * pj + mq, n))
            if j < NJ - 1:
                q += qk2(0, j + 1) + qk2(1, j + 1)
                # bc at blocks 3-4 (after reciprocal), c_proj 5-12, q by 13
                pops[j] = [0, 0, 0, 1, 1, 1, 1, 1, 1, 1, 1, 2, 2, 2, 0, 0]
            else:
                # last window: keep c_proj(j2) late so the PE stays warm
                # through the j3 normalize tail
                pops[j] = [0, 0, 0, 1, 1, 0, 0, 1, 1, 1, 1, 1, 1, 1, 1, 0]
            queues[j] = q

        # ---------- main emission ----------
        # HAM warm-up: dummy matmuls with no DMA dependency keep the PE
        # clock-gate busy while the first weight/activation DMAs land, so
        # the first real chains run at 2.4 GHz.
        dummy_in = const.tile([P, NQ], bf16)
        nc.vector.memset(dummy_in[:], 0.0)
        wps = ps_pool.tile([P, NQ], fp32, tag="st", bufs=1, name="warm")
        for _ in range(10):
            nc.tensor.matmul(wps[:], sel2[:, 0:P], dummy_in[:], start=True, stop=True)

        # pre-attention: minimum projections for (j=0, i=0)
        for fn in qk2(2, 0) + qk2(0, 0) + qk2(3, 0) + qk2(1, 0) + vc2(0):
            fn()

        alloc_pv(0)
        for j in range(NJ):
            queue = queues[j]
            qi = 0
            for i in range(NI):
                emit_block(j, i)
                for _ in range(pops[j][i]):
                    if qi < len(queue):
                        queue[qi]()
                        qi += 1
            while qi < len(queue):
                queue[qi]()
                qi += 1
            finish_j(j)

        # tail: normalize + c_proj for the last j (no next window to hide in).
        # A few dummy matmuls keep the PE warm while the reciprocal runs.
        j = NJ - 1
        tail_wps = ps_pool.tile([P, NQ], fp32, tag="st", bufs=1, name="tailwarm")
        for _ in range(6):
            nc.tensor.matmul(
                tail_wps[:], sel2[:, 0:P], dummy_in[:], start=True, stop=True
            )
        for p in range(2):
            bc_mul(j, p)
        tags = ["st", "pv"]
        for u in range(8):
            mq, n = divmod(u, 2)
            cproj_chain(4 * j + mq, n, tag=tags[u % 2])


def _get_nc():
    if "nc" not in _CACHE:
        _CACHE["nc"] = _build()
    return _CACHE["nc"]


def _make_in_maps(x, W_attn, b_attn, W_proj):
    import ml_dtypes

    bf = ml_dtypes.bfloat16
    x = np.asarray(x, np.float32)
    W_attn = np.asarray(W_attn, np.float32)
    b_attn = np.asarray(b_attn, np.float32)
    W_proj = np.asarray(W_proj, np.float32)
    scale = 1.0 / np.sqrt(np.float32(HD))
    in_maps = []
    for c in range(NCORES):
        b, g = divmod(c, 4)
        sl = slice(FG * g, FG * (g + 1))
        wq = W_attn[:, sl] * scale
        wk = W_attn[:, H:][:, sl]
        wv = W_attn[:, 2 * H :][:, sl]
        in_maps.append(
            {
                "xT": np.ascontiguousarray(x[b].T).astype(bf),
                "wqkv": np.ascontiguousarray(
                    np.concatenate([wq, wk, wv], axis=1)
                ).astype(bf),
                "bqk": np.concatenate(
                    [b_attn[sl] * scale, b_attn[H:][sl]]
                ).astype(bf),
                "bv": np.ascontiguousarray(b_attn[2 * H :][sl]).astype(bf),
                "wp": np.ascontiguousarray(W_proj[sl, :]).astype(bf),
            }
        )
    return in_maps


def _gather(results, b_proj):
    b_proj = np.asarray(b_proj, np.float32)
    y = np.empty((B, T, H), np.float32)
    for b in range(B):
        acc = results[4 * b]["out"].astype(np.float32)
        for g in range(1, 4):
            acc = acc + results[4 * b + g]["out"].astype(np.float32)
        y[b] = acc + b_proj[None, :]
    return y


def run(x, W_attn, b_attn, W_proj, b_proj, trace=False):
    from concourse.bass_utils import run_bass_kernel_spmd

    nc = _get_nc()
    in_maps = _make_in_maps(x, W_attn, b_attn, W_proj)
    res = run_bass_kernel_spmd(nc, in_maps, list(range(NCORES)), trace=trace)
    return _gather(res.results, b_proj), res


def kernel(x, W_attn, b_attn, W_proj, b_proj):
    y, _ = run(x, W_attn, b_attn, W_proj, b_proj, trace=False)
    return y


# revision 33
# speedup vs baseline: 1.2057x; 1.2057x over previous
"""Causal self-attention (non-masked softmax path) for TRN2, 8 NeuronCores.

Sharding: 2-way data parallel over batch x 4-way tensor parallel over heads.
Core c handles batch b = c // 4, head group g = c % 4 (heads 4g..4g+3).
Each core computes its QKV projection slice, full attention for its 4 heads,
and the row-parallel c_proj partial; the host sums the 4 partials per batch
(the all-reduce of row-parallel tensor parallelism) and adds b_proj.

The schedule is built around the ScalarE exp roofline: softmax exp is
16.8M elements/core at 1 elem/cycle/lane @1.2GHz = ~142us, which exceeds
the PE matmul issue floor (~130us). So ACT runs *only* exp, back-to-back,
and every other engine's work is interleaved around it:

 - QKV projection and c_proj matmuls are emitted as "filler" units spread
   between attention blocks so the PE stays busy without ever making ACT
   wait (no serial projection phase up front, no c_proj phase at the end).
 - All PSUM->SBUF evacuation runs on DVE (plain tensor_copy); q/k biases
   are applied with K=1 rank-1 matmuls appended to the projection chains,
   v bias likewise (ones-row trick), so no ACT Identity copies remain.
 - PV is col-packed: 2 heads per PE pass (M=64 each at col groups 0/64),
   softmax denominators come from 4 col-packed M=1 ones-matmuls (one per
   32-wide col group) accumulated in a dedicated psum bank, landing at
   partitions {0,32,64,96} exactly where the reciprocal/broadcast wants
   them. Denominator matmuls are emitted 2 blocks late so the dg bank's
   reuse dependency (prev j's reciprocal) never stalls the PE FIFO.
 - 1/d is broadcast across the 64 head-dim partitions with a single
   selector matmul per head pair (sel2), then applied by DVE tensor_mul
   straight out of PSUM.

All matmuls take bf16 inputs (fp32 matmul on TRN2 runs 4x slower) and
accumulate in fp32 PSUM. c_proj partials are DMA'd out in bf16 (halves
output traffic; the host sums in fp32). Measured end-to-end error vs the
fp32 reference is ~7e-3 scale-relative.

PSUM budget (8 banks): S tiles 2x[128,1024] (4) | PV accum 2x[128,512]
(2, heads pair-packed in partitions) | denominators 1x[128,512] (1) |
streaming proj/c_proj/bc tile 1x[128,512] (1).
"""

import numpy as np

B, T, H, NH, HD = 2, 2048, 1024, 16, 64
P = 128
FG = 256          # features per head group (4 heads x 64)
NQ = 512          # Tq chunk (psum free dim)
NJ = T // NQ      # 4
NI = T // P       # 16 key chunks
KH = H // P       # 8 hidden chunks
NCORES = 8
W3 = 3 * FG       # 768, wqkv row width
DDELAY = 3        # blocks by which denominator matmuls trail their E tile

_CACHE = {}


def _build():
    import concourse.bacc as bacc
    import concourse.mybir as mybir
    import concourse.tile as tile

    fp32 = mybir.dt.float32
    bf16 = mybir.dt.bfloat16

    nc = bacc.Bacc("TRN2", debug=False)
    xT = nc.dram_tensor("xT", [H, T], bf16, kind="ExternalInput").ap()
    wqkv = nc.dram_tensor("wqkv", [H, W3], bf16, kind="ExternalInput").ap()
    bqk = nc.dram_tensor("bqk", [2 * FG], fp32, kind="ExternalInput").ap()
    bv = nc.dram_tensor("bv", [FG], bf16, kind="ExternalInput").ap()
    wp = nc.dram_tensor("wp", [FG, H], bf16, kind="ExternalInput").ap()
    out = nc.dram_tensor("out", [T, H], bf16, kind="ExternalOutput").ap()

    with tile.TileContext(nc) as tc:
        _emit(nc, tc, mybir, xT, wqkv, bqk, bv, wp, out)
    nc.compile()
    return nc


def _emit(nc, tc, mybir, xT, wqkv, bqk, bv, wp, out):
    from contextlib import ExitStack

    fp32 = mybir.dt.float32
    bf16 = mybir.dt.bfloat16
    Exp = mybir.ActivationFunctionType.Exp
    Copy = mybir.ActivationFunctionType.Copy

    with ExitStack() as ctx:
        pool = lambda name, bufs=1, space="SBUF": ctx.enter_context(
            tc.tile_pool(name=name, bufs=bufs, space=space)
        )

        const = pool("const")
        # warm the exp table before any real dependency exists
        wt = const.tile([1, 16], bf16)
        nc.vector.memset(wt[:], 0.0)
        we = const.tile([1, 16], bf16)
        nc.scalar.activation(we[:], wt[:], Exp)

        ones = const.tile([1, P], bf16)       # K=1 lhsT for v-bias broadcast
        nc.vector.memset(ones[:], 1.0)
        ones_col = const.tile([P, 1], bf16)   # denominator lhsT (M=1)
        nc.vector.memset(ones_col[:], 1.0)
        # sel2[k, p*128 + m] = 1 iff k == 32*(2p + (m>=64)); sel2-slice.T @ rcp
        # replicates head (2p+hh)'s 1/d row across its 64 y^T partitions.
        sel2 = const.tile([P, 2 * P], bf16)
        nc.vector.memset(sel2[:], 0.0)
        for pp in range(2):
            for hh in range(2):
                r = 32 * (2 * pp + hh)
                nc.vector.memset(
                    sel2[r : r + 1, pp * P + hh * 64 : pp * P + (hh + 1) * 64], 1.0
                )
        bqk_sb = const.tile([P, 4], fp32)     # per-partition bias per m chunk
        nc.sync.dma_start(bqk_sb[:], bqk.rearrange("(m p) -> p m", p=P))
        bv_sb = const.tile([1, FG], bf16)
        nc.sync.dma_start(bv_sb[:], bv.rearrange("(o f) -> o f", o=1))
        dummy_in = const.tile([P, NQ], bf16)  # rhs for HAM warm-up matmuls
        nc.vector.memset(dummy_in[:], 0.0)

        xt_sb = pool("xt").tile([P, KH * T], bf16)
        w_sb = pool("w").tile([P, KH * W3], bf16)
        wp_sb = pool("wp").tile([P, 2 * H], bf16)

        def dma_x(k, jq):
            nc.sync.dma_start(
                xt_sb[:, k * T + jq * NQ : k * T + (jq + 1) * NQ],
                xT[k * P : (k + 1) * P, jq * NQ : (jq + 1) * NQ],
            )

        # prefetch in first-use order; w rows stay full-width (1.5KB/row DMA
        # efficiency beats delivering the m=2 slice early on thin 256B rows)
        for k in range(KH):
            nc.sync.dma_start(
                w_sb[:, k * W3 : (k + 1) * W3], wqkv[k * P : (k + 1) * P, :]
            )
            dma_x(k, 0)
        for jq in range(1, NJ):
            for k in range(KH):
                dma_x(k, jq)
        for kk in range(2):
            nc.sync.dma_start(
                wp_sb[:, kk * H : (kk + 1) * H], wp[kk * P : (kk + 1) * P, :]
            )

        qk_sb = pool("qk").tile([P, 4 * T], bf16)   # feat chunks: q0 q1 k0 k1
        v_sb = pool("v").tile([P, NI * FG], bf16)   # [p, (t h c)] c=64, no pad
        v4 = v_sb.rearrange("p (t h c) -> p t h c", t=NI, h=4, c=64)
        y_sb = pool("y").tile([P, 2 * T], bf16)     # y^T, feat pair chunks x T

        # one psum pool, 8 banks exactly:
        #   tag s  = 2 x [128,1024] (4 banks)  S^T pair tiles -> exp
        #   tag pv = 2 x [128, 512] (2 banks)  PV accum, 2 heads col-packed
        #   tag dg = 1 x [128, 512] (1 bank)   denominators at rows {0,32,64,96}
        #   tag st = 1 x [128, 512] (1 bank)   streaming proj/c_proj/bc tile
        ps_pool = ctx.enter_context(tc.tile_pool(name="ps", bufs=2, space="PSUM"))
        epool = pool("e", bufs=8)  # E lifetime spans DDELAY+1 blocks x 2 tiles
        pvs_pool = pool("pvs", bufs=2)
        rcp_pool = pool("rcp", bufs=2)
        outp = pool("outp", bufs=3)

        # ---------- filler work units (run on PE/DVE between attention) ----
        # Units are split to ~1us of PE time each so a popped unit never
        # delays the next attention block's S matmuls by more than the
        # per-block ACT slack. Two-part units (a/b) hold their "st" psum
        # slot between parts; the flat per-j queues keep a/b adjacent so no
        # other "st" user can interleave.
        def qk_chain_a(m, jq):
            ps = ps_pool.tile([P, NQ], fp32, tag="st", bufs=1, name=f"qk{m}_{jq}")
            state[("qkps", m, jq)] = ps
            for k in range(KH // 2):
                nc.tensor.matmul(
                    ps[:],
                    w_sb[:, k * W3 + m * P : k * W3 + (m + 1) * P],
                    xt_sb[:, k * T + jq * NQ : k * T + (jq + 1) * NQ],
                    start=(k == 0),
                    stop=False,
                )

        def qk_chain_b(m, jq):
            ps = state.pop(("qkps", m, jq))
            for k in range(KH // 2, KH):
                nc.tensor.matmul(
                    ps[:],
                    w_sb[:, k * W3 + m * P : k * W3 + (m + 1) * P],
                    xt_sb[:, k * T + jq * NQ : k * T + (jq + 1) * NQ],
                    start=False,
                    stop=(k == KH - 1),
                )
            # evacuate + per-partition bias broadcast in one DVE op
            nc.vector.tensor_scalar_add(
                out=qk_sb[:, m * T + jq * NQ : m * T + (jq + 1) * NQ],
                in0=ps[:],
                scalar1=bqk_sb[:, m : m + 1],
            )

        def _v_half(ps, tt, half):
            t = 2 * tt + half
            for k in range(KH):
                nc.tensor.matmul(
                    ps[:, half * FG : (half + 1) * FG],
                    xt_sb[:, k * T + t * P : k * T + (t + 1) * P],
                    w_sb[:, k * W3 + 2 * FG : (k + 1) * W3],
                    start=(k == 0),
                    stop=False,
                )
            nc.tensor.matmul(  # += ones.T @ bv (bias broadcast over rows)
                ps[:, half * FG : (half + 1) * FG],
                ones[0:1, :],
                bv_sb[0:1, :],
                start=False,
                stop=True,
            )

        def v_chain_a(tt):
            ps = ps_pool.tile([P, NQ], fp32, tag="st", bufs=1, name=f"v{tt}")
            state[("vps", tt)] = ps
            _v_half(ps, tt, 0)

        def v_chain_b(tt):
            ps = state.pop(("vps", tt))
            _v_half(ps, tt, 1)
            nc.vector.tensor_copy(v_sb[:, 2 * tt * FG : (2 * tt + 2) * FG], ps[:])

        def cproj_chain(mq, n, tag="st", act_copy=False):
            ps = ps_pool.tile(
                [P, NQ], fp32, tag=tag, bufs=(2 if tag == "pv" else 1),
                name=f"c{mq}_{n}",
            )
            for kk in range(2):
                nc.tensor.matmul(
                    ps[:],
                    y_sb[:, kk * T + mq * P : kk * T + (mq + 1) * P],
                    wp_sb[:, kk * H + n * NQ : kk * H + (n + 1) * NQ],
                    start=(kk == 0),
                    stop=(kk == 1),
                )
            ot = outp.tile([P, NQ], bf16, tag="o")
            if act_copy:  # ACT is idle in the tail; DVE is the last-mile limit
                nc.scalar.activation(ot[:], ps[:], Copy)
            else:
                nc.vector.tensor_copy(ot[:], ps[:])
            nc.sync.dma_start(out[mq * P : (mq + 1) * P, n * NQ : (n + 1) * NQ], ot[:])

        def dummy_unit():
            # three no-dependency matmuls: keep the PE HAM clock-gate warm
            # through schedule lulls (j boundaries, reciprocal tails)
            ps = ps_pool.tile([P, NQ], fp32, tag="st", bufs=1, name="dmy")
            for _ in range(3):
                nc.tensor.matmul(
                    ps[:], sel2[:, 0:P], dummy_in[:], start=True, stop=True
                )

        # ---------- per-j normalize state ----------
        state = {}

        def bc_mul(j, p):
            # broadcast 1/d across 64 partitions per head (one selector MM),
            # then y^T = pv * bc on DVE straight out of psum.
            rcp_t = state[("rcp", j)]
            pvs = state[("pvs", j, p)]
            bc = ps_pool.tile([P, NQ], fp32, tag="st", bufs=1, name=f"bc{j}_{p}")
            nc.tensor.matmul(
                bc[:], sel2[:, p * P : (p + 1) * P], rcp_t[:], start=True, stop=True
            )
            nc.vector.tensor_mul(
                y_sb[:, p * T + j * NQ : p * T + (j + 1) * NQ], pvs[:], bc[:]
            )

        # ---------- attention block emission ----------
        def _pv_pass(j, pi):
            for p in range(2):
                pv = state[("pv", j, p)]
                e = state[("e", j, pi, p)]
                for hh in range(2):
                    # 2 heads col-packed per pass share one psum bank, so they
                    # form ONE accumulation group: start clears has_written
                    # bank-wide; later first-writes overwrite-where-unwritten.
                    nc.tensor.matmul(
                        pv[64 * hh : 64 * (hh + 1), :],
                        v4[:, pi, 2 * p + hh, :],
                        e[:, hh * NQ : (hh + 1) * NQ],
                        start=(pi == 0 and hh == 0),
                        stop=(pi == NI - 1 and hh == 1),
                        tile_position=(0, 64 * hh),
                        skip_group_check=True,
                    )

        def emit_block(j, i):
            # delayed denominator matmuls for block (j, i-DDELAY); PV runs a
            # block late too so the j-boundary pv memsets never head-block
            # the PE FIFO.
            di = i - DDELAY
            if di >= 0:
                if di == 0:
                    dg = ps_pool.tile([P, NQ], fp32, tag="dg", bufs=1, name=f"dg{j}")
                    state[("dg", j)] = dg
                    # junk rows stay finite (1.0) so reciprocal never sees 0;
                    # accumulated rows start from 0.0 so the h>0 chains (which
                    # may accumulate-onto rather than overwrite, depending on
                    # how start's has_written clear scopes) are exact.
                    nc.vector.memset(dg[:], 1.0)
                    for h in (1, 2, 3):
                        nc.vector.memset(dg[32 * h : 32 * h + 1, :], 0.0)
                _denoms(j, di)
            for p in range(2):
                sp = ps_pool.tile([P, 2 * NQ], fp32, tag="s", name=f"s{j}_{i}_{p}")
                for hh in range(2):
                    bp = 64 * hh
                    nc.tensor.matmul(  # S^T chunk, K=64 row-packed x2
                        sp[:, hh * NQ : (hh + 1) * NQ],
                        qk_sb[
                            bp : bp + 64,
                            (2 + p) * T + i * P : (2 + p) * T + (i + 1) * P,
                        ],
                        qk_sb[bp : bp + 64, p * T + j * NQ : p * T + (j + 1) * NQ],
                        start=True,
                        stop=True,
                        tile_position=(bp, 0),
                    )
                e = epool.tile([P, 2 * NQ], bf16, tag="e")
                nc.scalar.activation(e[:], sp[:], Exp)
                state[("e", j, i, p)] = e
            if i >= 1:
                _pv_pass(j, i - 1)

        def _denoms(j, di):
            dg = state[("dg", j)]
            for h in range(4):
                p, hh = divmod(h, 2)
                e = state.pop(("e", j, di, p)) if hh == 1 else state[("e", j, di, p)]
                nc.tensor.matmul(  # d[32h, q] += sum_k E[k, q]; M=1 col-packed
                    dg[32 * h : 32 * h + 1, :],
                    ones_col[:, 0:1],
                    e[:, hh * NQ : (hh + 1) * NQ],
                    start=(di == 0 and h == 0),
                    stop=(di == NI - 1 and h == 3),
                    tile_position=(0, 32 * h),
                    skip_group_check=True,
                )

        def alloc_pv(j):
            for p in range(2):
                pv = ps_pool.tile([P, NQ], fp32, tag="pv", bufs=2, name=f"pv{j}_{p}")
                state[("pv", j, p)] = pv
                # partitions 64-127 are first written by a start=False matmul;
                # zero them so accumulate-onto-stale is exact either way.
                nc.vector.memset(pv[64:128, :], 0.0)

        def finish_j(j):
            # trailing PV pass + denominator matmuls (no delay needed past
            # block 15), then stage PV psum to SBUF (frees pv banks) +
            # reciprocal.
            _pv_pass(j, NI - 1)
            for di in range(NI - DDELAY, NI):
                _denoms(j, di)
            for p in range(2):
                pvs = pvs_pool.tile([P, NQ], fp32, tag="pvs", name=f"pvs{j}_{p}")
                nc.vector.tensor_copy(pvs[:], state.pop(("pv", j, p))[:])
                state[("pvs", j, p)] = pvs
            # next j's pv memsets go on the DVE queue BEFORE the (slow)
            # reciprocal, else PV(j+1, i=0) stalls the PE FIFO ~4us and the
            # HAM re-throttles the clock at every j boundary.
            if j + 1 < NJ:
                alloc_pv(j + 1)
            rcp_t = rcp_pool.tile([P, NQ], bf16, tag="rcp")
            with nc.allow_low_precision(reason="softmax denom broadcast in bf16"):
                nc.vector.reciprocal(rcp_t[:], state.pop(("dg", j))[:])
            state[("rcp", j)] = rcp_t

        # ---------- filler schedule ----------
        # Per-j ordered unit queues + per-block pop counts. Order guarantees
        # dependencies (a before b, producers a couple of blocks before
        # consumers) and keeps "st"-slot users strictly sequential.
        def qk2(m, jq):
            return [lambda: qk_chain_a(m, jq), lambda: qk_chain_b(m, jq)]

        def vc2(tt):
            return [lambda: v_chain_a(tt), lambda: v_chain_b(tt)]

        queues = {
            0: (
                vc2(1) + qk2(2, 1) + qk2(3, 1) + vc2(2) + vc2(3)
                + qk2(2, 2) + qk2(3, 2) + vc2(4) + vc2(5)
                + qk2(2, 3) + qk2(3, 3) + vc2(6) + vc2(7)
                + qk2(0, 1) + qk2(1, 1) + [dummy_unit, dummy_unit]
            ),
        }
        pops = {0: [2] * 16}
        for j in range(1, NJ):
            pj = j - 1
            q = [dummy_unit]
            if j < NJ - 1:
                q += qk2(0, j + 1) + qk2(1, j + 1)
            else:
                q += [dummy_unit]
            q += [lambda pj=pj: bc_mul(pj, 0), lambda pj=pj: bc_mul(pj, 1)]
            for u in range(8):
                mq, n = divmod(u, 2)
                q.append(lambda pj=pj, mq=mq, n=n: cproj_chain(4 * pj + mq, n))
            q += [dummy_unit]
            if j < NJ - 1:
                # dummy fills the boundary, q chains early (no rcp dep),
                # bc after the reciprocal lands, then c_proj, dummy at end
                pops[j] = [1, 1, 1, 1, 1, 1, 1, 1, 1, 1, 1, 1, 1, 1, 1, 1]
            else:
                pops[j] = [1, 1, 0, 1, 1, 1, 1, 1, 1, 1, 1, 1, 1, 0, 1, 1]
            queues[j] = q

        # ---------- main emission ----------
        # Pre-attention: minimum projections for (j=0, i=0). Dummy units
        # between chain parts keep the PE clock-gate warm while each part's
        # weight/activation DMAs land (the ramp is DMA-paced).
        pre = qk2(2, 0) + qk2(0, 0) + qk2(3, 0) + qk2(1, 0) + vc2(0)
        dummy_unit()
        dummy_unit()
        for u, fn in enumerate(pre):
            fn()
            # only after part-b units: a/b chain parts hold the st slot, a
            # dummy between them would collide with the held tile
            if u % 2 == 1 and u < 8:
                dummy_unit()

        alloc_pv(0)
        for j in range(NJ):
            queue = queues[j]
            qi = 0
            for i in range(NI):
                emit_block(j, i)
                for _ in range(pops[j][i]):
                    if qi < len(queue):
                        queue[qi]()
                        qi += 1
            while qi < len(queue):
                queue[qi]()
                qi += 1
            finish_j(j)

        # tail: normalize + c_proj for the last j (no next window to hide in).
        # Dummy matmuls keep the PE warm while the reciprocal runs; the
        # output copies ride the now-idle ACT engine so DVE isn't the last
        # serial resource.
        j = NJ - 1
        for _ in range(4):
            dummy_unit()
        for p in range(2):
            bc_mul(j, p)
        tags = ["st", "pv"]
        for u in range(8):
            mq, n = divmod(u, 2)
            cproj_chain(4 * j + mq, n, tag=tags[u % 2], act_copy=True)


def _get_nc():
    if "nc" not in _CACHE:
        _CACHE["nc"] = _build()
    return _CACHE["nc"]


def _make_in_maps(x, W_attn, b_attn, W_proj):
    import ml_dtypes

    bf = ml_dtypes.bfloat16
    x = np.asarray(x, np.float32)
    W_attn = np.asarray(W_attn, np.float32)
    b_attn = np.asarray(b_attn, np.float32)
    W_proj = np.asarray(W_proj, np.float32)
    scale = 1.0 / np.sqrt(np.float32(HD))
    in_maps = []
    for c in range(NCORES):
        b, g = divmod(c, 4)
        sl = slice(FG * g, FG * (g + 1))
        wq = W_attn[:, sl] * scale
        wk = W_attn[:, H:][:, sl]
        wv = W_attn[:, 2 * H :][:, sl]
        in_maps.append(
            {
                "xT": np.ascontiguousarray(x[b].T).astype(bf),
                "wqkv": np.ascontiguousarray(
                    np.concatenate([wq, wk, wv], axis=1)
                ).astype(bf),
                "bqk": np.concatenate(
                    [b_attn[sl] * scale, b_attn[H:][sl]]
                ).astype(np.float32),
                "bv": np.ascontiguousarray(b_attn[2 * H :][sl]).astype(bf),
                "wp": np.ascontiguousarray(W_proj[sl, :]).astype(bf),
            }
        )
    return in_maps


def _gather(results, b_proj):
    b_proj = np.asarray(b_proj, np.float32)
    y = np.empty((B, T, H), np.float32)
    for b in range(B):
        acc = results[4 * b]["out"].astype(np.float32)
        for g in range(1, 4):
            acc = acc + results[4 * b + g]["out"].astype(np.float32)
        y[b] = acc + b_proj[None, :]
    return y


def run(x, W_attn, b_attn, W_proj, b_proj, trace=False):
    from concourse.bass_utils import run_bass_kernel_spmd

    nc = _get_nc()
    in_maps = _make_in_maps(x, W_attn, b_attn, W_proj)
    res = run_bass_kernel_spmd(nc, in_maps, list(range(NCORES)), trace=trace)
    return _gather(res.results, b_proj), res


def kernel(x, W_attn, b_attn, W_proj, b_proj):
    y, _ = run(x, W_attn, b_attn, W_proj, b_proj, trace=False)
    return y
